# revision 1
# baseline (speedup 1.0000x reference)
"""Trainium2 Bass kernel for the 3-branch custom attention module.

Math (per batch b, head h):
  q,k,v = x @ W? + b?   (heads of dim 64)
  s12[i,j] = q_i . k_j ;  p1 = softmax_j(s12), p2 = softmax_j(s12/8)
  s3[i,j]  = sum_d Ws[d] tanh(k_id + q_jd) ; p3 = softmax_j(s3)
  out = ((p1+p2+p3)/3) @ v

The (N,N,64) tanh intermediate is never materialized: tanh(k+q) is
expanded in a 4-frequency sine series fitted on [-5.3, 5.3]
    tanh(t) ~= sum_m c_m sin(w_m t)
and each sin(w_m(k+q)) = sin(w_m k)cos(w_m q) + cos(w_m k)sin(w_m q)
turns branch 3 into a K=512 matmul of per-element sin/cos basis maps.
ACT Sin is only valid on [-pi, pi]; frequencies whose args can leave
that range are range-reduced in turns: y = (+-w/2pi) x + shift_turns,
u = y - int(y) (TRN2 fp->int converts round-to-nearest-even, so
u in [-1/2, 1/2]), then sin(2 pi u) == sin(+-w x + shift) exactly by
periodicity.

Sharding: 48 (b,h) pairs -> 8 cores, 6 heads each (core c: batch c//2,
heads 6*(c%2)..+6). Each core loads x[b]^T and the 384 weight columns
of its heads, computes its (197, 384) slice of the output.
"""

import os
import sys
from contextlib import ExitStack

import numpy as np

sys.path.insert(0, "/opt/trn_rl_repo")

import concourse.bass as bass
import concourse.tile as tile
from concourse import bacc, mybir

AF = mybir.ActivationFunctionType
ALU = mybir.AluOpType
AX = mybir.AxisListType
F32 = mybir.dt.float32
F16 = mybir.dt.float16
BF16 = mybir.dt.bfloat16
I32 = mybir.dt.int32

_e = os.environ.get
TS_ENG = _e("K_TS", "gp")        # ts_i32 engine: gp|dve
QBS_ENG = _e("K_QBS", "gp")      # qbs mul engine: gp|dve
PSUMCFG = _e("K_PSUM", "222")    # pw/pt/cp bufs
ODMA = _e("K_ODMA", "4")         # output DMA split: 2|4
HWB = int(_e("K_HWB", "3"))      # hwork pool bufs
SSPL = int(_e("K_SSPL", "1"))    # split basis sin/mul into column halves
B, N, D, H, HD = 4, 197, 768, 12, 64
NCORES = 8
HPC = 6                 # heads per core
COLS = HPC * HD         # 384
PI = float(np.pi)
P1 = N - 128            # 69
KT = D // 128           # 6 contraction tiles for projections
NM = 2                  # sine terms
FB = HPC * N            # 1182

FREQS = [0.48465577, 1.85504882]
COEFS = [1.25950049, 0.19128238]
# FREQS[0]*max|k| + pi/2 < pi -> no range reduction needed for m=0.

# cvec per-partition constant columns (see CV_* indices):
#   wsc m: c_m*[Ws;Ws] | radian biases k/q | pm-omega/2pi k/q (mod freqs)
#   | turn shifts k/q
CV_WSC = 0                    # .. NM-1
CV_RAD_K = NM                 # [0]*64+[pi/2]*64   (k halves = [sin; cos])
CV_RAD_Q = NM + 1             # [pi/2]*64+[0]*64   (q halves = [cos; sin])
CV_PM_K = NM + 2              # .. +NM-2: [+w/2pi]*64+[-w/2pi]*64
CV_PM_Q = CV_PM_K + (NM - 1)  # [-w/2pi]*64+[+w/2pi]*64
CV_TS_K = CV_PM_Q + (NM - 1)  # [0]*64+[0.25]*64
CV_TS_Q = CV_TS_K + 1         # [0.25]*64+[0]*64
CV_N = CV_TS_Q + 1


def _w_dram_view(ap):
    """(768, 384) DRAM tensor viewed as one (128, 6*384) SBUF-shaped AP:
    tile[p, kt*384 + c] = W[kt*128 + p, c]."""
    return bass.AP(
        tensor=ap.tensor,
        offset=ap.offset,
        ap=[[COLS, 128], [128 * COLS, KT], [1, COLS]],
    )


def _xt_dram_view(ap):
    """(768, 197) DRAM tensor viewed as (128, 6*197):
    tile[p, kt*197 + i] = xT[kt*128 + p, i]."""
    return bass.AP(
        tensor=ap.tensor,
        offset=ap.offset,
        ap=[[N, 128], [128 * N, KT], [1, N]],
    )


def _build_body(ctx, tc, aps):
    nc = tc.nc
    out_ap = aps["out"]

    consts = ctx.enter_context(tc.tile_pool(name="consts", bufs=1))
    ones_col = consts.tile([128, 1], F16)
    nc.vector.memset(ones_col, 1.0)
    ones_row = consts.tile([1, N], F16)
    nc.vector.memset(ones_row, 1.0)
    neg11_row = consts.tile([1, 128], F16)
    nc.vector.memset(neg11_row, -11.0)
    bm025 = consts.tile([128, 1], F32)
    nc.vector.memset(bm025, -0.025)
    cv = consts.tile([128, CV_N], F32)
    nc.sync.dma_start(cv, aps["cvec"])
    bq_sb = consts.tile([128, 3], F32)
    nc.sync.dma_start(bq_sb, aps["bq"])
    bk_sb = consts.tile([128, 3], F32)
    nc.sync.dma_start(bk_sb, aps["bk"])
    bv3_sb = consts.tile([128, COLS], F32)
    bv3 = aps["bv3"]
    nc.scalar.dma_start(
        bv3_sb, bass.AP(tensor=bv3.tensor, offset=bv3.offset, ap=[[0, 128]] + bv3.ap)
    )

    # ---- batched input loads, spread across engine sequencers ----
    xp = ctx.enter_context(tc.tile_pool(name="x", bufs=1))
    DSPL = int(_e("K_DSPL", "2"))    # input DMA split halves
    xt_all = xp.tile([128, FB], F16, tag="xt")       # [p, kt*197 + i]
    xv = _xt_dram_view(aps["xt"])
    hk = KT // DSPL
    for d in range(DSPL):
        nc.sync.dma_start(
            xt_all[:, d * hk * N:(d + 1) * hk * N],
            bass.AP(tensor=xv.tensor, offset=xv.offset + d * hk * 128 * N,
                    ap=[xv.ap[0], [128 * N, hk], [1, N]]),
        )
    w_all = {}
    for nm, eng in (("wq", nc.gpsimd), ("wk", nc.scalar), ("wv", nc.sync)):
        w_all[nm] = xp.tile([128, KT * COLS], F16, tag=nm, name=nm)
        wv_ = _w_dram_view(aps[nm])
        for d in range(DSPL):
            eng.dma_start(
                w_all[nm][:, d * hk * COLS:(d + 1) * hk * COLS],
                bass.AP(tensor=wv_.tensor, offset=wv_.offset + d * hk * 128 * COLS,
                        ap=[wv_.ap[0], [128 * COLS, hk], [1, COLS]]),
            )

    # ---- projections ----
    qkv = ctx.enter_context(tc.tile_pool(name="qkv", bufs=1))
    qT = [qkv.tile([128, N], F16, tag=f"qT{mt}", name=f"qT{mt}") for mt in range(3)]
    kTt = [qkv.tile([128, N], F16, tag=f"kT{mt}", name=f"kT{mt}") for mt in range(3)]
    VC = HD + 1  # 65: per-head v columns + ones column
    vsb = [qkv.tile([128, HPC * VC], F16, tag=f"v{pt}", name=f"v{pt}") for pt in range(2)]
    out_sb = [
        qkv.tile([128, COLS], F32, tag=f"osb{it}", name=f"osb{it}") for it in range(2)
    ]
    with tc.tile_pool(name="pproj", bufs=1, space="PSUM") as pp:
        for wname, bias_sb, dst in (("wq", bq_sb, qT), ("wk", bk_sb, kTt)):
            for mt in range(3):
                ps = pp.tile([128, N], F32, tag=f"p{wname}{mt}")
                for kt in range(KT):
                    nc.tensor.matmul(
                        ps,
                        w_all[wname][:, kt * COLS + mt * 128: kt * COLS + (mt + 1) * 128],
                        xt_all[:, kt * N:(kt + 1) * N],
                        start=(kt == 0),
                        stop=(kt == KT - 1),
                    )
                nc.vector.tensor_scalar_add(dst[mt], ps, bias_sb[:, mt:mt + 1])
        for pt2 in range(2):
            np_ = 128 if pt2 == 0 else P1
            ps = pp.tile([128, COLS], F32, tag=f"pv{pt2}")
            for kt in range(KT):
                nc.tensor.matmul(
                    ps[:np_, :],
                    xt_all[:, kt * N + pt2 * 128: kt * N + pt2 * 128 + np_],
                    w_all["wv"][:, kt * COLS:(kt + 1) * COLS],
                    start=(kt == 0),
                    stop=(kt == KT - 1),
                )
            vview = vsb[pt2][:np_, :].rearrange("p (h c) -> p h c", c=VC)
            nc.vector.tensor_add(
                vview[:, :, 0:HD],
                ps[:np_, :].rearrange("p (h c) -> p h c", c=HD),
                bv3_sb[:np_, :].rearrange("p (h c) -> p h c", c=HD),
            )
            nc.vector.memset(vview[:, :, HD:VC], 1.0)

    # ---- d-major duplicated layouts: rows = [d (head-major cols); same] ----
    bp = ctx.enter_context(tc.tile_pool(name="basis", bufs=1))
    kdup = bp.tile([128, FB], F16, tag="kdup")
    qdup = bp.tile([128, FB], F16, tag="qdup")
    for h in range(HPC):
        po = (h % 2) * 64
        ek = nc.gpsimd if h % 2 == 0 else nc.vector
        eq = nc.vector if h % 2 == 0 else nc.gpsimd
        ek.tensor_copy(kdup[0:64, h * N:(h + 1) * N], kTt[h // 2][po:po + 64, :])
        eq.tensor_copy(qdup[0:64, h * N:(h + 1) * N], qT[h // 2][po:po + 64, :])
    nc.gpsimd.tensor_copy(kdup[64:128, :], kdup[0:64, :])
    nc.vector.tensor_copy(qdup[64:128, :], qdup[0:64, :])

    # ---- sine/cosine basis maps ----
    kb = [bp.tile([128, FB], BF16, tag=f"kb{m}", name=f"kb{m}") for m in range(NM)]
    qbs = [bp.tile([128, FB], BF16, tag=f"qbs{m}", name=f"qbs{m}") for m in range(NM)]
    mwp = ctx.enter_context(tc.tile_pool(name="modwork", bufs=1))
    for m in range(NM):
        HB = FB // SSPL
        if m == 0:
            for d in range(SSPL):
                cs = slice(d * HB, (d + 1) * HB)
                nc.scalar.activation(kb[0][:, cs], kdup[:, cs], AF.Sin,
                                     bias=cv[:, CV_RAD_K:CV_RAD_K + 1], scale=FREQS[0])
            qb = mwp.tile([128, FB], BF16, tag="qb")
            for d in range(SSPL):
                cs = slice(d * HB, (d + 1) * HB)
                nc.scalar.activation(qb[:, cs], qdup[:, cs], AF.Sin,
                                     bias=cv[:, CV_RAD_Q:CV_RAD_Q + 1], scale=FREQS[0])
                (nc.gpsimd if QBS_ENG == "gp" else nc.vector).tensor_scalar_mul(
                    qbs[0][:, cs], qb[:, cs], cv[:, 0:1])
        else:
            j = m - 1
            for dup, pmcol, shcol, is_q in (
                (kdup, CV_PM_K + j, CV_RAD_K, False),
                (qdup, CV_PM_Q + j, CV_RAD_Q, True),
            ):
                # n = rint((+-w/2pi) x + shift_turns)  (DVE int32 write rints)
                # u' = (+-w/2pi) x - n;  sin(2pi u' + shift_rad) is the true
                # sin/cos by 2pi-periodicity, with |2pi u' + shift| <= pi.
                cid = f"{m}{int(is_q)}"
                ni = mwp.tile([128, FB], I32, tag=f"ni{cid}", name=f"ni{cid}")
                (nc.gpsimd if TS_ENG == "gp" else nc.vector).tensor_scalar(
                    out=ni,
                    in0=dup,
                    scalar1=cv[:, pmcol:pmcol + 1],
                    scalar2=cv[:, CV_TS_K + (shcol - CV_RAD_K):CV_TS_K + (shcol - CV_RAD_K) + 1],
                    op0=ALU.mult,
                    op1=ALU.add,
                )
                w2 = mwp.tile([128, FB], F32, tag=f"w2{cid}", name=f"w2{cid}")
                nc.vector.scalar_tensor_tensor(
                    w2, dup, cv[:, pmcol:pmcol + 1], ni,
                    op0=ALU.mult, op1=ALU.subtract,
                )
                if is_q:
                    qb = mwp.tile([128, FB], BF16, tag=f"qb{cid}", name=f"qb{cid}")
                    for d in range(SSPL):
                        cs = slice(d * HB, (d + 1) * HB)
                        nc.scalar.activation(
                            qb[:, cs], w2[:, cs], AF.Sin,
                            bias=cv[:, shcol:shcol + 1], scale=2.0 * PI
                        )
                        (nc.gpsimd if QBS_ENG == "gp" else nc.vector).tensor_scalar_mul(
                            qbs[m][:, cs], qb[:, cs], cv[:, m:m + 1])
                else:
                    for d in range(SSPL):
                        cs = slice(d * HB, (d + 1) * HB)
                        nc.scalar.activation(
                            kb[m][:, cs], w2[:, cs], AF.Sin,
                            bias=cv[:, shcol:shcol + 1], scale=2.0 * PI
                        )

    # ---- per-head attention (transposed maps; no P transpose) ----
    # spT[j, i] holds [s12^T - 11 | s3^T]; fp16 exps go straight to SBUF;
    # ctx_b and its softmax denominator come out of one matmul against
    # [v | ones]; normalization is a per-partition scale on the (i, 64)
    # ctx tiles.
    hw = ctx.enter_context(tc.tile_pool(name="hwork", bufs=HWB))
    pw = ctx.enter_context(tc.tile_pool(name="psw", bufs=int(PSUMCFG[0]), space="PSUM"))
    pcx = ctx.enter_context(tc.tile_pool(name="psc", bufs=int(PSUMCFG[1]), space="PSUM"))
    for h in range(HPC):
        po = (h % 2) * 64
        qTh = qT[h // 2][po:po + 64, :]
        kTh = kTt[h // 2][po:po + 64, :]
        ews = []
        for jt in range(2):
            njt = 128 if jt == 0 else P1
            sp = pw.tile([128, 2 * N], F32, tag=f"sp{jt}", name=f"sp{jt}_{h}")
            # s12^T[j, i] = k_j . q_i, then -11 via rank-1 (fp16 exp range)
            nc.tensor.matmul(
                sp[:njt, 0:N],
                kTh[:, jt * 128:jt * 128 + njt],
                qTh,
                start=True,
                stop=False,
            )
            nc.tensor.matmul(
                sp[:njt, 0:N],
                neg11_row[:, 0:njt],
                ones_row,
                start=False,
                stop=True,
            )
            # s3^T[j, i]: q-basis as stationary, k-basis moving
            for m in range(NM):
                nc.tensor.matmul(
                    sp[:njt, N:2 * N],
                    qbs[m][:, h * N + jt * 128: h * N + jt * 128 + njt],
                    kb[m][:, h * N:(h + 1) * N],
                    start=(m == 0),
                    stop=(m == NM - 1),
                )
            ew = hw.tile([128, 2 * N], F16, tag=f"ew{jt}", name=f"ew{jt}_{h}")
            e2 = hw.tile([128, N], F16, tag=f"e2{jt}", name=f"e2{jt}_{h}")
            nc.scalar.activation(ew[:njt, :], sp[:njt, :], AF.Exp)
            # e2 = exp(s12/8 - 1.4) = exp((s12-11)/8 - 0.025)
            nc.scalar.activation(e2[:njt, :], sp[:njt, 0:N], AF.Exp,
                                 scale=0.125, bias=bm025[:njt, :])
            ews.append((ew, e2))
        for it in range(2):
            nit = 128 if it == 0 else P1
            cpk = pcx.tile([128, 3 * VC], F32, tag=f"cp{it}", name=f"cp{it}_{h}")
            for b in range(3):
                for jt in range(2):
                    njt = 128 if jt == 0 else P1
                    ew, e2 = ews[jt]
                    if b == 0:
                        eslice = ew[:njt, it * 128:it * 128 + nit]
                    elif b == 1:
                        eslice = e2[:njt, it * 128:it * 128 + nit]
                    else:
                        eslice = ew[:njt, N + it * 128:N + it * 128 + nit]
                    nc.tensor.matmul(
                        cpk[:nit, b * VC:(b + 1) * VC],
                        eslice,
                        vsb[jt][:njt, h * VC:(h + 1) * VC],
                        start=(jt == 0),
                        stop=(jt == 1),
                    )
            rr = hw.tile([128, 3], F32, tag=f"rr{it}", name=f"rr{it}_{h}")
            dview = cpk[:nit, :].rearrange("p (b c) -> p b c", c=VC)[:, :, HD:VC]
            nc.vector.reciprocal(rr[:nit, :], dview.rearrange("p b c -> p (b c)"))
            acc = hw.tile([128, HD], F32, tag=f"acc{it}", name=f"acc{it}_{h}")
            nc.vector.tensor_scalar_mul(
                acc[:nit, :], cpk[:nit, 0:HD], rr[:nit, 0:1]
            )
            nc.vector.scalar_tensor_tensor(
                acc[:nit, :], cpk[:nit, VC:VC + HD], rr[:nit, 1:2], acc[:nit, :],
                op0=ALU.mult, op1=ALU.add,
            )
            nc.vector.scalar_tensor_tensor(
                out_sb[it][:nit, h * HD:(h + 1) * HD],
                cpk[:nit, 2 * VC:2 * VC + HD], rr[:nit, 2:3], acc[:nit, :],
                op0=ALU.mult, op1=ALU.add,
            )
    if ODMA == "12":
        # stream each head's columns out as soon as both row-blocks are done
        for h in range(HPC):
            for it in range(2):
                nit = 128 if it == 0 else P1
                eng = nc.sync if (h + it) % 2 == 0 else nc.scalar
                eng.dma_start(
                    out_ap[it * 128:it * 128 + nit, h * HD:(h + 1) * HD],
                    out_sb[it][:nit, h * HD:(h + 1) * HD],
                )
    elif ODMA == "4":
        # column-halves depend only on heads 0-2 / 3-5, so the first pair
        # overlaps the tail heads
        HC = 3 * HD  # 192
        nc.sync.dma_start(out_ap[0:128, 0:HC], out_sb[0][:128, 0:HC])
        nc.scalar.dma_start(out_ap[128:N, 0:HC], out_sb[1][:P1, 0:HC])
        nc.sync.dma_start(out_ap[0:128, HC:COLS], out_sb[0][:128, HC:COLS])
        nc.scalar.dma_start(out_ap[128:N, HC:COLS], out_sb[1][:P1, HC:COLS])
    else:
        nc.sync.dma_start(out_ap[0:128, :], out_sb[0][:128, :])
        nc.scalar.dma_start(out_ap[128:N, :], out_sb[1][:P1, :])


def build_nc():
    nc = bacc.Bacc(
        "TRN2",
        target_bir_lowering=False,
        debug=False,
        enable_asserts=True,
        num_devices=NCORES,
    )
    aps = {}
    aps["xt"] = nc.dram_tensor("xt", (D, N), F16, kind="ExternalInput").ap()
    for nm in ("wq", "wk", "wv"):
        aps[nm] = nc.dram_tensor(nm, (D, COLS), F16, kind="ExternalInput").ap()
    aps["bq"] = nc.dram_tensor("bq", (128, 3), F32, kind="ExternalInput").ap()
    aps["bk"] = nc.dram_tensor("bk", (128, 3), F32, kind="ExternalInput").ap()
    aps["bv3"] = nc.dram_tensor("bv3", (COLS,), F32, kind="ExternalInput").ap()
    aps["cvec"] = nc.dram_tensor("cvec", (128, CV_N), F32, kind="ExternalInput").ap()
    aps["out"] = nc.dram_tensor("out", (N, COLS), F32, kind="ExternalOutput").ap()
    with tile.TileContext(nc) as tc:
        with ExitStack() as ctx:
            _build_body(ctx, tc, aps)
    nc.compile()
    return nc


def make_in_maps(inputs):
    """Slice full inputs into the 8 per-core input maps."""
    hs = np.asarray(inputs["hidden_states"], np.float32)
    Ws = np.asarray(inputs["Ws"], np.float32)
    wsc2 = np.concatenate([Ws, Ws])
    cvec = np.zeros((128, CV_N), np.float32)
    for m in range(NM):
        cvec[:, m] = np.float32(COEFS[m]) * wsc2
    cvec[64:128, CV_RAD_K] = PI / 2
    cvec[0:64, CV_RAD_Q] = PI / 2
    for j, m in enumerate(range(1, NM)):
        wt = np.float32(FREQS[m] / (2 * PI))
        cvec[0:64, CV_PM_K + j] = wt
        cvec[64:128, CV_PM_K + j] = -wt
        cvec[0:64, CV_PM_Q + j] = -wt
        cvec[64:128, CV_PM_Q + j] = wt
    cvec[64:128, CV_TS_K] = 0.25
    cvec[0:64, CV_TS_Q] = 0.25
    in_maps = []
    for c in range(NCORES):
        b = c // 2
        hb = c % 2
        cols = slice(hb * COLS, (hb + 1) * COLS)
        m = {
            "xt": np.ascontiguousarray(hs[b].T).astype(np.float16),
            "wq": np.ascontiguousarray(inputs["Wq"][:, cols]).astype(np.float16),
            "wk": np.ascontiguousarray(inputs["Wk"][:, cols]).astype(np.float16),
            "wv": np.ascontiguousarray(
                np.asarray(inputs["Wv"], np.float32)[:, cols] / np.float32(3.0)
            ).astype(np.float16),
            "bq": np.ascontiguousarray(
                np.asarray(inputs["bq"], np.float32)[cols].reshape(3, 128).T
            ),
            "bk": np.ascontiguousarray(
                np.asarray(inputs["bk"], np.float32)[cols].reshape(3, 128).T
            ),
            "bv3": np.ascontiguousarray(
                np.asarray(inputs["bv"], np.float32)[cols] / np.float32(3.0)
            ),
            "cvec": cvec,
        }
        in_maps.append(m)
    return in_maps


def assemble(results):
    out = np.zeros((B, N, D), np.float32)
    for c in range(NCORES):
        b = c // 2
        hb = c % 2
        out[b, :, hb * COLS:(hb + 1) * COLS] = results[c]["out"]
    return out


_NC_CACHE = {}


def kernel(**inputs):
    import jax

    try:
        jax.config.update("jax_compilation_cache_dir", "/tmp/jax_neff_cache")
        jax.config.update("jax_persistent_cache_min_compile_time_secs", 1.0)
    except Exception:
        pass
    from concourse import bass_utils

    if "nc" not in _NC_CACHE:
        _NC_CACHE["nc"] = build_nc()
    nc = _NC_CACHE["nc"]
    in_maps = make_in_maps(inputs)
    res = bass_utils.run_bass_kernel_spmd(nc, in_maps, core_ids=list(range(NCORES)))
    return assemble(res.results)

